# revision 1
# baseline (speedup 1.0000x reference)
# CondConv2d Trainium2 kernel.
#
# Math (per sample n=(b,l)):
#   pooled[c]   = mean_{h,w} x[n,c,h,w]
#   allxet      = [p0,p0,p0,p1,p2,p3] temporal window (first frame dup'd twice)
#   calib[c,t]  = conv1d(allxet, tconv_w)[c,t] + tconv_b[c]
#   gate[t]     = conv1d(allxet, fc_w)[0,t] + fc_b
#   scale[n,c]  = calib[c,l] + 1
#   out[n,o]    = conv2d(x[n] * scale[n,:,None,None], weight) + bias[o]*(gate[l]+1)
# (the per-sample weight scale fw = weight * scale[n,ci] is folded into the
#  input because conv is linear in each input channel)
#
# Sharding: data-parallel over b: 8 cores x 2 samples. Weights replicated.
# Conv as implicit GEMM: contraction over ci (2 chunks of 128 partitions),
# 9 shifted-window matmuls accumulate in PSUM; fp32 data streamed as
# float32r (full-rate on TRN2 for N>=256).

import numpy as np


def _install_axon_ntff_shim():
    # This container's `antenv` stub lacks `axon_hooks`, which
    # bass_utils imports unconditionally when trace=True under axon.
    # Provide it (and register the ctypes NTFF hook if the .so is
    # present) so tracing works; missing pieces degrade to no-trace.
    import os
    import sys
    import types

    try:
        import antenv.axon_hooks  # noqa: F401

        return
    except Exception:
        pass
    try:
        import antenv
    except Exception:
        return
    mod = types.ModuleType("antenv.axon_hooks")
    mod._hook = None

    def set_axon_ntff_profile_hook(h):
        mod._hook = h

    def get_axon_ntff_profile_hook():
        return mod._hook

    mod.set_axon_ntff_profile_hook = set_axon_ntff_profile_hook
    mod.get_axon_ntff_profile_hook = get_axon_ntff_profile_hook
    sys.modules["antenv.axon_hooks"] = mod
    antenv.axon_hooks = mod
    try:
        from trn_agent_boot.trn_boot import _ntff_profile_via_ctypes

        so = "/opt/axon/libaxon_pjrt.so"
        if os.path.exists(so):
            mod._hook = _ntff_profile_via_ctypes(so)
    except Exception:
        pass


_install_axon_ntff_shim()

import concourse.bass as bass
import concourse.tile as tile
from concourse import mybir
from concourse.bass_utils import run_bass_kernel_spmd

B, L, CIN, COUT, KS, H, W = 16, 4, 256, 256, 3, 32, 32
NCORES = 8
BS = B // NCORES      # batch samples per core
CC = CIN // 128       # ci chunks
OC = COUT // 128      # co chunks
WP = W + 2            # x tile row width incl. zero pad cols
FP32 = mybir.dt.float32
FP32R = mybir.dt.float32r
HHALF = 16            # psum bank = 512 fp32 = 16 rows of 32

_last_results = None  # test harness reads exec_time_ns from here


def _split_excess_waits(nc):
    # walrus in this toolchain encodes exactly one sem wait per engine
    # instruction (TPB_EVENTS has a single wait slot) and optimize_sems
    # is disabled, so Tile can emit instructions with >1 wait that fail
    # codegen ("Too many sync wait commands").  Split the excess waits
    # into standalone EventSemaphore instructions on the same engine
    # stream immediately before the instruction; in-order issue makes
    # this equivalent.  Applies to Drain too (CTRL struct: one wait).
    n = 0
    f = nc.m.functions[0]
    for bb in f.blocks:
        insts = list(bb.instructions)
        out = []
        changed = False
        for inst in insts:
            si = inst.sync_info
            if si is not None:
                waits = list(si.on_wait)
                if len(waits) > 1:
                    for w in waits[:-1]:
                        n += 1
                        es = mybir.InstEventSemaphore(name=f"ES-SPLIT-{n}")
                        es.engine = inst.engine
                        es.sync_info = mybir.SyncInfo(on_wait=[w], on_update=[])
                        out.append(es)
                    si.on_wait = [waits[-1]]
                    inst.sync_info = si
                    changed = True
            out.append(inst)
        if changed:
            bb.instructions = out
    return n


def build_nc():
    nc = bass.Bass()
    x_d = nc.dram_tensor("x", [BS, L, CIN, H, W], FP32, kind="ExternalInput")
    w_d = nc.dram_tensor("w", [128, CC, 9, COUT], FP32, kind="ExternalInput")
    tcw_d = nc.dram_tensor("tconv", [128, CC, 3, CIN], FP32, kind="ExternalInput")
    fcw_d = nc.dram_tensor("fc", [128, CC, 3], FP32, kind="ExternalInput")
    bias_d = nc.dram_tensor("bias2", [128, OC], FP32, kind="ExternalInput")
    tb_d = nc.dram_tensor("tb", [128, CC], FP32, kind="ExternalInput")
    fcb_d = nc.dram_tensor("fcb", [1, 1], FP32, kind="ExternalInput")
    out_d = nc.dram_tensor("out", [BS, L, COUT, H, W], FP32, kind="ExternalOutput")

    with tile.TileContext(nc) as tc:
        with (
            tc.tile_pool(name="singles", bufs=1) as singles,
            tc.tile_pool(name="xraw", bufs=12) as xraw,
            tc.tile_pool(name="outp", bufs=6) as outp,
            tc.tile_pool(name="pp_conv", bufs=2, space="PSUM") as pp_conv,
            tc.tile_pool(name="pp_c", bufs=2, space="PSUM") as pp_c,
            tc.tile_pool(name="pp_g", bufs=1, space="PSUM") as pp_g,
            tc.tile_pool(name="pp_gb", bufs=1, space="PSUM") as pp_gb,
        ):
            # ---- persistent params ----
            w_sb = singles.tile([128, CC, 9, COUT], FP32, tag="w")
            w_raw = singles.tile([128, CC, 9, COUT], FP32, tag="w_raw")
            nc.gpsimd.dma_start(out=w_raw[:], in_=w_d[:])
            # round the conv weights to FP32r once (required by the fp32r
            # matmul path; a copy with fp32r output is the rounding op)
            nc.vector.tensor_copy(w_sb[:].bitcast(FP32R), w_raw[:])
            # 1/(H*W) pooling normalization is folded into the conv1d
            # weights; the scaled tiles are written by DVE only so the
            # matmuls that consume them carry a single wait condition
            tcw_raw = singles.tile([128, CC, 3, CIN], FP32, tag="tcw_raw")
            nc.gpsimd.dma_start(out=tcw_raw[:], in_=tcw_d[:])
            tcw_sb = singles.tile([128, CC, 3, CIN], FP32, tag="tcw")
            nc.vector.tensor_scalar_mul(tcw_sb[:], tcw_raw[:], 1.0 / (H * W))
            fcw_raw = singles.tile([128, CC, 3], FP32, tag="fcw_raw")
            nc.gpsimd.dma_start(out=fcw_raw[:], in_=fcw_d[:])
            fcw_sb = singles.tile([128, CC, 3], FP32, tag="fcw")
            nc.vector.tensor_scalar_mul(fcw_sb[:], fcw_raw[:], 1.0 / (H * W))
            bias_sb = singles.tile([128, OC], FP32, tag="bias")
            nc.gpsimd.dma_start(out=bias_sb[:], in_=bias_d[:])
            tb_sb = singles.tile([128, CC], FP32, tag="tb")
            nc.gpsimd.dma_start(out=tb_sb[:], in_=tb_d[:])
            fcb_sb = singles.tile([1, 1], FP32, tag="fcb")
            nc.gpsimd.dma_start(out=fcb_sb[:], in_=fcb_d[:])

            tb1_sb = singles.tile([128, CC], FP32, tag="tb1")
            nc.vector.tensor_scalar_add(tb1_sb[:], tb_sb[:], 1.0)   # tconv_b + 1
            fcb1_sb = singles.tile([1, 1], FP32, tag="fcb1")
            nc.vector.tensor_scalar_add(fcb1_sb[:], fcb_sb[:], 1.0)  # fc_b + 1
            ones_sb = singles.tile([1, 128], FP32, tag="ones")
            nc.vector.memset(ones_sb[:], 1.0)
            zcol_sb = singles.tile([128, H, 1], FP32, tag="zcol")
            nc.vector.memset(zcol_sb[:], 0.0)

            # ---- persistent per-sample state ----
            allxet = singles.tile([128, CC, BS, L + 2], FP32, tag="allxet")
            s_sb = singles.tile([128, CC, BS, L], FP32, tag="s")
            g_sb = singles.tile([1, BS, L], FP32, tag="g")
            fb_sb = singles.tile([128, BS, L, OC], FP32, tag="fb")

            # fp32r conv input tiles: only ever written by rounding ops
            # (fp32r memset for the zero-pad cols, fp32r ACT scale for data)
            x_t = {}
            for b in range(BS):
                for l in range(L):
                    for ci in range(CC):
                        xt = singles.tile([128, H, WP], FP32R, tag=f"x{b}_{l}_{ci}")
                        x_t[(b, l, ci)] = xt
                        # fp32r memset fails walrus' ISA check; a copy
                        # with fp32r out is the supported rounding/zero op
                        nc.vector.tensor_copy(xt[:, :, 0:1], zcol_sb[:])
                        nc.vector.tensor_copy(xt[:, :, WP - 1:WP], zcol_sb[:])

            x_r = {}
            for b in range(BS):
                # ---- load x (staging, fp32), pool spatial sums ----
                for l in range(L):
                    for ci in range(CC):
                        xr = xraw.tile([128, H, W], FP32, tag="xr")
                        x_r[(b, l, ci)] = xr
                        nc.gpsimd.dma_start(
                            out=xr[:],
                            in_=x_d[b, l, ci * 128:(ci + 1) * 128, :, :],
                        )
                        nc.vector.reduce_sum(
                            out=allxet[:, ci, b, 2 + l:3 + l],
                            in_=xr[:],
                            axis=mybir.AxisListType.XY,
                        )
                # duplicate first frame twice
                for ci in range(CC):
                    nc.vector.tensor_copy(allxet[:, ci, b, 0:1], allxet[:, ci, b, 2:3])
                    nc.vector.tensor_copy(allxet[:, ci, b, 1:2], allxet[:, ci, b, 2:3])

                # ---- calib: per-frame channel scales ----
                for oc in range(OC):
                    pc = pp_c.tile([128, L], FP32, tag="pc")
                    mms = [(ci, k) for ci in range(CC) for k in range(3)]
                    for i, (ci, k) in enumerate(mms):
                        nc.tensor.matmul(
                            pc[:, :],
                            lhsT=tcw_sb[:, ci, k, oc * 128:(oc + 1) * 128],
                            rhs=allxet[:, ci, b, k:k + L],
                            start=(i == 0),
                            stop=(i == len(mms) - 1),
                        )
                    # scale = calib + tconv_b + 1 ; channel index of scale ==
                    # output channel of tconv, so oc chunk == ci chunk here
                    nc.vector.tensor_scalar_add(
                        s_sb[:, oc, b, :], pc[:, :], tb1_sb[:, oc:oc + 1]
                    )

                # ---- gate -> per-sample bias ----
                pg = pp_g.tile([128, L], FP32, tag="pg")
                mms = [(ci, k) for ci in range(CC) for k in range(3)]
                for i, (ci, k) in enumerate(mms):
                    nc.tensor.matmul(
                        pg[0:1, :],
                        lhsT=fcw_sb[:, ci, k:k + 1],
                        rhs=allxet[:, ci, b, k:k + L],
                        start=(i == 0),
                        stop=(i == len(mms) - 1),
                    )
                nc.vector.tensor_scalar_add(
                    g_sb[0:1, b, :], pg[0:1, :], fcb1_sb[0:1, 0:1]
                )
                # broadcast (gate+fc_b+1) across partitions via rank-1 matmul
                gb = pp_gb.tile([128, L], FP32, tag="gb")
                nc.tensor.matmul(
                    gb[:, :], lhsT=ones_sb[0:1, :], rhs=g_sb[0:1, b, :],
                    start=True, stop=True,
                )
                for l in range(L):
                    for oc in range(OC):
                        nc.vector.tensor_mul(
                            fb_sb[:, b, l, oc:oc + 1],
                            gb[:, l:l + 1],
                            bias_sb[:, oc:oc + 1],
                        )

                # ---- scale input channels in place ----
                # scale doubles as the FP32r rounding op for the matmul rhs
                for l in range(L):
                    for ci in range(CC):
                        nc.scalar.mul(
                            x_t[(b, l, ci)][:, :, 1:W + 1],
                            x_r[(b, l, ci)][:],
                            s_sb[:, ci, b, l:l + 1],
                        )

                # ---- the conv: implicit GEMM ----
                for l in range(L):
                    for oc in range(OC):
                        ps = pp_conv.tile([128, H, W], FP32, tag="convps")
                        for half in range(H // HHALF):
                            h0 = half * HHALF
                            group = []
                            for ci in range(CC):
                                for kh in range(3):
                                    dh = kh - 1
                                    hA = max(h0, -dh)
                                    hB = min(h0 + HHALF, H - dh)
                                    if hB <= hA:
                                        continue
                                    for kw in range(3):
                                        group.append((ci, kh, kw, hA, hB))
                            for i, (ci, kh, kw, hA, hB) in enumerate(group):
                                dh = kh - 1
                                lhsT = w_sb[
                                    :, ci, kh * 3 + kw, oc * 128:(oc + 1) * 128
                                ].bitcast(FP32R)
                                rhs = x_t[(b, l, ci)][
                                    :, hA + dh:hB + dh, kw:kw + W
                                ].bitcast(FP32R)
                                nc.tensor.matmul(
                                    ps[:, hA:hB, :],
                                    lhsT=lhsT,
                                    rhs=rhs,
                                    start=(i == 0),
                                    stop=(i == len(group) - 1),
                                )
                        osb = outp.tile([128, H, W], FP32, tag="osb")
                        nc.vector.tensor_scalar_add(
                            osb[:], ps[:], fb_sb[:, b, l, oc:oc + 1]
                        )
                        nc.gpsimd.dma_start(
                            out=out_d[b, l, oc * 128:(oc + 1) * 128, :, :],
                            in_=osb[:],
                        )
    _split_excess_waits(nc)
    return nc


def kernel(x, weight, bias, tconv_w, tconv_b, fc_w, fc_b):
    global _last_results
    x = np.ascontiguousarray(np.asarray(x, dtype=np.float32))
    weight = np.asarray(weight, dtype=np.float32)
    bias = np.asarray(bias, dtype=np.float32)
    tconv_w = np.asarray(tconv_w, dtype=np.float32)
    tconv_b = np.asarray(tconv_b, dtype=np.float32)
    fc_w = np.asarray(fc_w, dtype=np.float32)
    fc_b = np.asarray(fc_b, dtype=np.float32)

    # host-side layout packing (shared across cores)
    w_host = np.ascontiguousarray(
        weight.transpose(1, 2, 3, 0).reshape(CC, 128, 9, COUT).transpose(1, 0, 2, 3)
    )
    tcw_host = np.ascontiguousarray(
        tconv_w.transpose(1, 2, 0).reshape(CC, 128, 3, CIN).transpose(1, 0, 2, 3)
    )
    fcw_host = np.ascontiguousarray(
        fc_w[0].reshape(CC, 128, 3).transpose(1, 0, 2)
    )
    bias_host = np.ascontiguousarray(bias.reshape(OC, 128).T)
    tb_host = np.ascontiguousarray(tconv_b.reshape(CC, 128).T)
    fcb_host = np.ascontiguousarray(fc_b.reshape(1, 1))

    nc = build_nc()
    in_maps = []
    for core in range(NCORES):
        in_maps.append({
            "x": np.ascontiguousarray(x[core * BS:(core + 1) * BS]),
            "w": w_host,
            "tconv": tcw_host,
            "fc": fcw_host,
            "bias2": bias_host,
            "tb": tb_host,
            "fcb": fcb_host,
        })
    res = run_bass_kernel_spmd(nc, in_maps, core_ids=list(range(NCORES)))
    _last_results = res
    out = np.concatenate(
        [r["out"].reshape(BS * L, COUT, H, W) for r in res.results], axis=0
    )
    return out

